# revision 1
# baseline (speedup 1.0000x reference)
"""Trainium2 Bass kernel for nn_Block_68719476955 (dense transformer block).

Math: with H=1 the attention softmax is over a singleton axis, so
attn_prob == 1.0 exactly and the whole attention reduces to
x @ w_kv + b_kv (w_attn / b_attn / mask do not affect the output).

The block computed per token row x_t (E=2048):
    t  = x @ w_kv + b_kv
    h  = LN(x + t) * g1 + b1
    u  = gelu(h @ w_fc + b_fc)          # exact gelu
    v  = u @ w_mproj + b_mproj
    out= LN(v + x) * g2 + b2

Distribution: pure data-parallel over the 8192 tokens across 8 cores
(1024 tokens/core), full weights on every core, no collectives.

Device layout: feature-major ("transposed") activations [E, tokens] so
every matmul runs with the weight block as the stationary operand
(lhsT = W[in,out] chunk, rhs = activation [in, tok]) and the output
lands feature-major again — zero on-device transposes. LayerNorm
reductions (over features = partitions) run on the TensorEngine as
ones-vector matmuls, software-pipelined one chunk behind the producing
matmuls; per-token stats come back across partitions via
gpsimd.partition_broadcast (no PE involvement).

Precision: bf16 matmul operands, fp32 PSUM accumulation, fp32
residual adds and final normalize.
"""

import numpy as np
import ml_dtypes
from contextlib import ExitStack

import concourse.bacc as bacc
import concourse.mybir as mybir
import concourse.tile as tile
from concourse.bass_utils import run_bass_kernel_spmd

P = 128
B, S, E = 4, 2048, 2048
H4 = 4 * E                 # 8192 mlp hidden
NCORES = 8
TOK = (B * S) // NCORES    # 1024 tokens per core
T = 512                    # token tile (2 per core)
NT = TOK // T
EO = E // P                # 16
FO = H4 // P               # 64
HC = 4                     # hidden chunks for the mlp (2048 features each)
HCO = FO // HC             # 16 m-blocks per hidden chunk
LN_EPS = 1e-5

F32 = mybir.dt.float32
BF16 = mybir.dt.bfloat16
AF = mybir.ActivationFunctionType
ALU = mybir.AluOpType

_CACHED_NC = None


def _build():
    nc = bacc.Bacc(None, target_bir_lowering=False)

    xf_d = nc.dram_tensor("xf", [E, TOK], F32, kind="ExternalInput")
    xb_d = nc.dram_tensor("xb", [E, TOK], BF16, kind="ExternalInput")
    wkv_d = nc.dram_tensor("wkv", [EO, P, EO, P], BF16, kind="ExternalInput")
    wfc_d = nc.dram_tensor("wfc", [FO, P, EO, P], BF16, kind="ExternalInput")
    wmp_d = nc.dram_tensor("wmp", [EO, P, FO, P], BF16, kind="ExternalInput")
    bkv_d = nc.dram_tensor("bkv", [P, EO], F32, kind="ExternalInput")
    bfc_d = nc.dram_tensor("bfc", [P, FO], F32, kind="ExternalInput")
    bmp_d = nc.dram_tensor("bmp", [P, EO], F32, kind="ExternalInput")
    g1_d = nc.dram_tensor("g1", [P, EO], F32, kind="ExternalInput")
    b1_d = nc.dram_tensor("b1", [P, EO], F32, kind="ExternalInput")
    g2_d = nc.dram_tensor("g2", [P, EO], F32, kind="ExternalInput")
    b2_d = nc.dram_tensor("b2", [P, EO], F32, kind="ExternalInput")
    out_d = nc.dram_tensor("out", [E, TOK], F32, kind="ExternalOutput")

    with tile.TileContext(nc) as tc, ExitStack() as ctx:
        consts = ctx.enter_context(tc.tile_pool(name="consts", bufs=1))
        xbp = ctx.enter_context(tc.tile_pool(name="xbp", bufs=1))
        wp = ctx.enter_context(tc.tile_pool(name="wp", bufs=3))
        xcp = ctx.enter_context(tc.tile_pool(name="xcp", bufs=2))
        rbp = ctx.enter_context(tc.tile_pool(name="rbp", bufs=2))
        up = ctx.enter_context(tc.tile_pool(name="up", bufs=1))
        vp = ctx.enter_context(tc.tile_pool(name="vp", bufs=1))
        tmp = ctx.enter_context(tc.tile_pool(name="tmp", bufs=4))
        sqp = ctx.enter_context(tc.tile_pool(name="sqp", bufs=8))
        stp = ctx.enter_context(tc.tile_pool(name="stp", bufs=1))
        bcp = ctx.enter_context(tc.tile_pool(name="bcp", bufs=2))
        psmm = ctx.enter_context(tc.tile_pool(name="psmm", bufs=4, space="PSUM"))
        psst = ctx.enter_context(tc.tile_pool(name="psst", bufs=2, space="PSUM"))

        # x in bf16, one tile per 128-feature chunk so the kv matmuls can
        # start as soon as their chunk lands (16 parallel DMAs, issued first).
        xbs = []
        for k in range(EO):
            xk = xbp.tile([P, TOK], BF16, tag=f"xb{k}")
            eng = nc.gpsimd if k % 2 == 0 else nc.scalar
            eng.dma_start(xk[:], xb_d[k * P:(k + 1) * P, :])
            xbs.append(xk)

        # --- constants (gpsimd queue keeps Sync free for the weight stream) ---
        bkv_t = consts.tile([P, EO], F32)
        nc.gpsimd.dma_start(bkv_t[:], bkv_d[:, :])
        bfc_t = consts.tile([P, FO], F32)
        nc.gpsimd.dma_start(bfc_t[:], bfc_d[:, :])
        bmp_t = consts.tile([P, EO], F32)
        nc.gpsimd.dma_start(bmp_t[:], bmp_d[:, :])
        g1_t = consts.tile([P, EO], F32)
        nc.gpsimd.dma_start(g1_t[:], g1_d[:, :])
        b1_t = consts.tile([P, EO], F32)
        nc.gpsimd.dma_start(b1_t[:], b1_d[:, :])
        g2_t = consts.tile([P, EO], F32)
        nc.gpsimd.dma_start(g2_t[:], g2_d[:, :])
        b2_t = consts.tile([P, EO], F32)
        nc.gpsimd.dma_start(b2_t[:], b2_d[:, :])
        ones_col = consts.tile([P, 1], BF16)
        nc.vector.memset(ones_col[:], 1.0)
        eps_t = consts.tile([1, 1], F32)
        nc.vector.memset(eps_t[:], LN_EPS)

        # warm the PE (HAM clock gate) while input DMAs are in flight
        warm_rhs = consts.tile([P, T], BF16)
        nc.vector.memset(warm_rhs[:], 1.0)
        warm_ps = psst.tile([1, T], F32, tag="pss")
        for _ in range(40):
            nc.tensor.matmul(warm_ps[:], lhsT=ones_col[:], rhs=warm_rhs[:],
                             start=True, stop=True)

        def tsl(t):
            return slice(t * T, (t + 1) * T)

        def ln_finalize(ps_sum, ps_sq):
            """per-token mean/var from accumulated sums -> [P,2,T] bcast
            (slot 0 = rstd, slot 1 = mean*rstd). PE-free."""
            st = stp.tile([1, 3, T], F32, tag="st")
            nc.vector.tensor_scalar_mul(st[:, 0, :], ps_sum[:], 1.0 / E)   # mean
            nc.vector.tensor_scalar_mul(st[:, 1, :], ps_sq[:], 1.0 / E)    # E[x^2]
            nc.vector.tensor_mul(out=st[:, 2, :], in0=st[:, 0, :], in1=st[:, 0, :])
            nc.vector.tensor_tensor(st[:, 1, :], st[:, 1, :], st[:, 2, :],
                                    ALU.subtract)                          # var
            nc.scalar.activation(st[:, 2, :], st[:, 1, :], AF.Sqrt,
                                 bias=eps_t[:], scale=1.0)                 # sqrt(var+eps)
            nc.vector.reciprocal(out=st[:, 2, :], in_=st[:, 2, :])         # rstd
            nc.vector.tensor_mul(out=st[:, 1, :], in0=st[:, 0, :], in1=st[:, 2, :])
            # slots: [1]=mean*rstd, [2]=rstd -> bcast adjacent pair
            bc = bcp.tile([P, 2, T], F32, tag="bc")
            nc.gpsimd.partition_broadcast(bc[:], st[:, 1:3, :])
            return bc

        # ---------- phase A: kv matmul + residual + LN1 stats ----------
        def phase_a(t, hook=None, block_cb=None):
            r1b = rbp.tile([P, EO, T], BF16, tag="rb")
            ps_sum = psst.tile([1, T], F32, tag="pss")
            ps_sq = psst.tile([1, T], F32, tag="psq")
            pending = []  # software-pipeline the stats MMs behind the k-loops

            def emit_stats(pi):
                # pair-sum chunks 2pi,2pi+1 on DVE, then one ones-MM per pair
                m0, m1 = 2 * pi, 2 * pi + 1
                pr = sqp.tile([P, T], BF16, tag="sq")
                nc.vector.tensor_add(out=pr[:], in0=r1b[:, m0, :],
                                     in1=r1b[:, m1, :])
                sq0 = pending.pop(0)
                sq1 = pending.pop(0)
                pq = sqp.tile([P, T], BF16, tag="sq")
                nc.vector.tensor_add(out=pq[:], in0=sq0[:], in1=sq1[:])
                nc.tensor.matmul(ps_sum[:], lhsT=ones_col[:], rhs=pr[:],
                                 start=(pi == 0), stop=(pi == EO // 2 - 1))
                nc.tensor.matmul(ps_sq[:], lhsT=ones_col[:], rhs=pq[:],
                                 start=(pi == 0), stop=(pi == EO // 2 - 1))

            for m in range(EO):
                wt = wp.tile([P, EO, P], BF16, tag="w")
                nc.sync.dma_start(wt[:], wkv_d[m])
                ps = psmm.tile([P, T], F32, tag="ps")
                for k in range(EO):
                    nc.tensor.matmul(ps[:], lhsT=wt[:, k, :],
                                     rhs=xbs[k][:, tsl(t)],
                                     start=(k == 0), stop=(k == EO - 1))
                if m == 0 and hook is not None:
                    hook()  # previous phase's deferred stats MMs
                xc = xcp.tile([P, T], F32, tag="xc")
                nc.sync.dma_start(xc[:], xf_d[m * P:(m + 1) * P, tsl(t)])
                t1 = tmp.tile([P, T], F32, tag="t1")
                nc.scalar.activation(t1[:], ps[:], AF.Identity,
                                     bias=bkv_t[:, m:m + 1], scale=1.0)
                nc.vector.tensor_add(out=r1b[:, m, :], in0=t1[:], in1=xc[:])
                sq = sqp.tile([P, T], BF16, tag="sq")
                nc.vector.tensor_mul(out=sq[:], in0=r1b[:, m, :], in1=r1b[:, m, :])
                pending.append(sq)
                if m % 4 == 3 and m < EO - 1:
                    emit_stats(m // 2 - 1)
                    emit_stats(m // 2)
                if block_cb is not None:
                    block_cb(m)

            def finish():
                emit_stats(EO // 2 - 2)
                emit_stats(EO // 2 - 1)
            return r1b, (ps_sum, ps_sq), finish

        def normalize1_chunk(r1b, bc, m):
            # in place: h overwrites r1b (WAR on the stats MMs is tracked)
            t1 = tmp.tile([P, T], F32, tag="t1")
            nc.vector.tensor_mul(out=t1[:], in0=r1b[:, m, :], in1=bc[:, 1, :])
            nc.vector.tensor_tensor(t1[:], t1[:], bc[:, 0, :], ALU.subtract)
            nc.scalar.activation(r1b[:, m, :], t1[:], AF.Identity,
                                 bias=b1_t[:, m:m + 1],
                                 scale=g1_t[:, m:m + 1])

        def normalize1(r1b, bc):
            for m in range(EO):
                normalize1_chunk(r1b, bc, m)
            return r1b

        # ---------- phase B: mlp; LN2 stats fused into last-chunk evicts ----
        def phase_b(t, h, hook=None):
            v2f = vp.tile([P, EO, T], F32, tag="v")     # r2 = v + b_mproj + x
            ps_sum = psst.tile([1, T], F32, tag="pss")
            ps_sq = psst.tile([1, T], F32, tag="psq")
            pending = []

            def emit_stats2(pi):
                r2c0, sq0 = pending.pop(0)
                r2c1, sq1 = pending.pop(0)
                pr = sqp.tile([P, T], BF16, tag="sq")
                nc.vector.tensor_add(out=pr[:], in0=r2c0[:], in1=r2c1[:])
                pq = sqp.tile([P, T], BF16, tag="sq")
                nc.vector.tensor_add(out=pq[:], in0=sq0[:], in1=sq1[:])
                nc.tensor.matmul(ps_sum[:], lhsT=ones_col[:], rhs=pr[:],
                                 start=(pi == 0), stop=(pi == EO // 2 - 1))
                nc.tensor.matmul(ps_sq[:], lhsT=ones_col[:], rhs=pq[:],
                                 start=(pi == 0), stop=(pi == EO // 2 - 1))

            u = up.tile([P, FO, T], BF16, tag="u")
            for ma in range(FO):
                wt = wp.tile([P, EO, P], BF16, tag="w")
                nc.sync.dma_start(wt[:], wfc_d[ma])
                ps = psmm.tile([P, T], F32, tag="ps")
                for k in range(EO):
                    nc.tensor.matmul(ps[:], lhsT=wt[:, k, :], rhs=h[:, k, :],
                                     start=(k == 0), stop=(k == EO - 1))
                if ma == 0 and hook is not None:
                    hook()  # previous phase's deferred stats + LN2 output
                nc.scalar.activation(u[:, ma, :], ps[:], AF.Gelu,
                                     bias=bfc_t[:, ma:ma + 1], scale=1.0)
            for mo in range(EO):
                ps = psmm.tile([P, T], F32, tag="ps")
                for hc in range(HC):
                    wt = wp.tile([P, HCO, P], BF16, tag="w")
                    nc.sync.dma_start(wt[:], wmp_d[mo][:, hc * HCO:(hc + 1) * HCO, :])
                    for k in range(HCO):
                        nc.tensor.matmul(ps[:], lhsT=wt[:, k, :],
                                         rhs=u[:, hc * HCO + k, :],
                                         start=(hc == 0 and k == 0),
                                         stop=(hc == HC - 1 and k == HCO - 1))
                xc = xcp.tile([P, T], F32, tag="xc")
                nc.gpsimd.dma_start(xc[:], xf_d[mo * P:(mo + 1) * P, tsl(t)])
                nc.scalar.activation(v2f[:, mo, :], ps[:], AF.Identity,
                                     bias=bmp_t[:, mo:mo + 1], scale=1.0)
                nc.vector.tensor_add(out=v2f[:, mo, :], in0=v2f[:, mo, :],
                                     in1=xc[:])
                r2c = sqp.tile([P, T], BF16, tag="sq")
                nc.vector.tensor_copy(out=r2c[:], in_=v2f[:, mo, :])
                sq = sqp.tile([P, T], BF16, tag="sq")
                nc.vector.tensor_mul(out=sq[:], in0=r2c[:], in1=r2c[:])
                pending.append((r2c, sq))
                if mo >= 2 and mo % 2 == 0:
                    emit_stats2(mo // 2 - 1)

            def finish():
                emit_stats2(EO // 2 - 1)
            return v2f, (ps_sum, ps_sq), finish

        # ---------- phase C: final normalize + output ----------
        def phase_c_out(t, v2f, bc, tail):
            dma_eng = nc.sync if tail else nc.gpsimd
            for m in range(EO):
                t1 = tmp.tile([P, T], F32, tag="t1")
                nc.vector.tensor_mul(out=t1[:], in0=v2f[:, m, :], in1=bc[:, 1, :])
                nc.vector.tensor_tensor(t1[:], t1[:], bc[:, 0, :], ALU.subtract)
                if tail:
                    nc.scalar.activation(t1[:], t1[:], AF.Identity,
                                         bias=b2_t[:, m:m + 1],
                                         scale=g2_t[:, m:m + 1])
                else:
                    nc.vector.tensor_scalar(t1[:], t1[:], g2_t[:, m:m + 1],
                                            b2_t[:, m:m + 1], ALU.mult, ALU.add)
                dma_eng.dma_start(out_d[m * P:(m + 1) * P, tsl(t)], t1[:])

        # Emission order interleaves the two token tiles so the PE never
        # waits on DVE normalize chains or LN finalize chains; each phase's
        # final stats MMs are deferred into the next phase's first block.
        r1b0, s0, f0 = phase_a(0)
        state = {}

        def hook_a1():
            f0()
            state["bc10"] = ln_finalize(*s0)

        def a1_block_cb(m):
            normalize1_chunk(r1b0, state["bc10"], m)

        r1b1, s1, f1 = phase_a(1, hook=hook_a1, block_cb=a1_block_cb)

        def hook_b0():
            f1()
            bc11 = ln_finalize(*s1)
            normalize1(r1b1, bc11)

        v0, s20, f20 = phase_b(0, r1b0, hook=hook_b0)

        def hook_b1():
            f20()
            bc20 = ln_finalize(*s20)
            phase_c_out(0, v0, bc20, tail=False)

        v1, s21, f21 = phase_b(1, r1b1, hook=hook_b1)
        f21()
        bc21 = ln_finalize(*s21)
        phase_c_out(1, v1, bc21, tail=True)

    nc.compile()
    return nc


def _get_nc():
    global _CACHED_NC
    if _CACHED_NC is None:
        _CACHED_NC = _build()
    return _CACHED_NC


def _prep_inputs(x, w_kv, b_kv, w_fc, b_fc, w_mproj, b_mproj,
                 ln1_g, ln1_b, ln2_g, ln2_b):
    """Host-side shard + retile. Returns per-core input maps."""
    bf = ml_dtypes.bfloat16
    x_flat = np.ascontiguousarray(np.asarray(x, dtype=np.float32).reshape(B * S, E))

    # weights: [in, out] -> [m, p, o, c] tiles, in = o*128+p, out = m*128+c
    def retile(w, io, oo):
        w = np.asarray(w, dtype=np.float32).reshape(io, P, oo, P)
        return np.ascontiguousarray(w.transpose(2, 1, 0, 3)).astype(bf)

    wkv_t = retile(w_kv, EO, EO)
    wfc_t = retile(w_fc, EO, FO)
    wmp_t = retile(w_mproj, FO, EO)

    def p2d(v):  # [n*P] -> [P, n] with chunk o in column o
        v = np.asarray(v, np.float32)
        return np.ascontiguousarray(v.reshape(-1, P).T)

    shared = {
        "wkv": wkv_t, "wfc": wfc_t, "wmp": wmp_t,
        "bkv": p2d(b_kv), "bfc": p2d(b_fc), "bmp": p2d(b_mproj),
        "g1": p2d(ln1_g), "b1": p2d(ln1_b), "g2": p2d(ln2_g), "b2": p2d(ln2_b),
    }
    in_maps = []
    for c in range(NCORES):
        xT = np.ascontiguousarray(x_flat[c * TOK:(c + 1) * TOK].T)  # [E, TOK] f32
        in_maps.append({"xf": xT, "xb": xT.astype(bf), **shared})
    return in_maps


def _run(inputs, trace=False):
    nc = _get_nc()
    in_maps = _prep_inputs(
        inputs["x"], inputs["w_kv"], inputs["b_kv"], inputs["w_fc"],
        inputs["b_fc"], inputs["w_mproj"], inputs["b_mproj"],
        inputs["ln1_g"], inputs["ln1_b"], inputs["ln2_g"], inputs["ln2_b"])
    res = run_bass_kernel_spmd(nc, in_maps, core_ids=list(range(NCORES)),
                               trace=trace)
    outs = [np.asarray(res.results[c]["out"], dtype=np.float32).T
            for c in range(NCORES)]
    full = np.concatenate(outs, axis=0).reshape(B, S, E)
    return full, res


def kernel(**inputs) -> np.ndarray:
    out, _ = _run(inputs, trace=False)
    return out



# revision 14
# speedup vs baseline: 1.2303x; 1.2303x over previous
"""Trainium2 Bass kernel for nn_Block_68719476955 (dense transformer block).

Math: with H=1 the attention softmax is over a singleton axis, so
attn_prob == 1.0 exactly and the whole attention reduces to
x @ w_kv + b_kv (w_attn / b_attn / mask do not affect the output).

Folded main path: with A = I + w_kv and C = I - 11^T/E (mean-centering),
    LN1(x@A + b_kv) @ w_fc  ==  rstd ⊙ (x @ W1c + cb2) + c0
where W1c = A @ C @ (diag(g1) w_fc) is precomputed on host, cb2/c0 are
constant vectors, and rstd is the per-token LN1 inverse std.  So the
E x E kv matmul disappears from the main bf16 path; it is needed only
to produce the LN1 *variance*, which tolerates fp8: that matmul runs as
fp8e4 (TRN e4m3) DoubleRow matmuls at ~2x column rate.

Block per token row (E=2048):
    r1   = x @ A + b_kv              # fp8 DoubleRow, feeds only mean/var
    alpha= rsqrt(var(r1) + eps)
    u    = gelu(alpha*(x @ W1c + cb2) + c0)     # bf16 matmul
    v    = u @ w_mproj + b_mproj                # bf16 matmul
    out  = LN(v + x) * g2 + b2

Distribution: pure data-parallel over the 8192 tokens across 8 cores
(1024 tokens/core), full weights on every core, no collectives.

Device layout: feature-major ("transposed") activations [E, tokens];
every matmul has the weight block stationary.  LayerNorm feature
reductions run on the TensorEngine as ones-vector matmuls over
quad-summed chunks; per-token stats broadcast back across partitions
via gpsimd.partition_broadcast.
"""

import numpy as np
import ml_dtypes
from contextlib import ExitStack

import concourse.bacc as bacc
import concourse.mybir as mybir
import concourse.tile as tile
from concourse.bass_utils import run_bass_kernel_spmd

P = 128
B, S, E = 4, 2048, 2048
H4 = 4 * E                 # 8192 mlp hidden
NCORES = 8
TOK = (B * S) // NCORES    # 1024 tokens per core
T = 512                    # token tile (2 per core)
NT = TOK // T
EO = E // P                # 16
EP = EO // 2               # 8 fp8 k-pairs
FO = H4 // P               # 64
HC = 4                     # hidden chunks for the mlp (2048 features each)
HCO = FO // HC             # 16 m-blocks per hidden chunk
LN_EPS = 1e-5
F8MAX = 240.0

F32 = mybir.dt.float32
BF16 = mybir.dt.bfloat16
F8 = mybir.dt.float8e4
AF = mybir.ActivationFunctionType
ALU = mybir.AluOpType
DR = mybir.MatmulPerfMode.DoubleRow

_CACHED_NC = {}


def _build(has_cb2: bool):
    nc = bacc.Bacc(None, target_bir_lowering=False)

    xf_d = nc.dram_tensor("xf", [E, TOK], F32, kind="ExternalInput")
    xb_d = nc.dram_tensor("xb", [E, TOK], BF16, kind="ExternalInput")
    x8_d = nc.dram_tensor("x8", [EP, P, 2, TOK], F8, kind="ExternalInput")
    a8_d = nc.dram_tensor("a8", [EO, P, EP, 2, P], F8, kind="ExternalInput")
    w1_d = nc.dram_tensor("w1", [FO, P, EO, P], BF16, kind="ExternalInput")
    wmp_d = nc.dram_tensor("wmp", [EO, P, FO, P], BF16, kind="ExternalInput")
    bkv_d = nc.dram_tensor("bkv", [P, EO], F32, kind="ExternalInput")
    c0_d = nc.dram_tensor("c0", [P, FO], F32, kind="ExternalInput")
    cb2_d = nc.dram_tensor("cb2", [P, FO], F32, kind="ExternalInput")
    bmp_d = nc.dram_tensor("bmp", [P, EO], F32, kind="ExternalInput")
    g2_d = nc.dram_tensor("g2", [P, EO], F32, kind="ExternalInput")
    b2_d = nc.dram_tensor("b2", [P, EO], F32, kind="ExternalInput")
    sc1_d = nc.dram_tensor("sc1", [P, 1], F32, kind="ExternalInput")
    out_d = nc.dram_tensor("out", [E, TOK], F32, kind="ExternalOutput")

    with tile.TileContext(nc) as tc, ExitStack() as ctx:
        consts = ctx.enter_context(tc.tile_pool(name="consts", bufs=1))
        xbp = ctx.enter_context(tc.tile_pool(name="xbp", bufs=1))
        x8p = ctx.enter_context(tc.tile_pool(name="x8p", bufs=1))
        a8p = ctx.enter_context(tc.tile_pool(name="a8p", bufs=3))
        wp = ctx.enter_context(tc.tile_pool(name="wp", bufs=3))
        r1p = ctx.enter_context(tc.tile_pool(name="r1p", bufs=5))
        xcp = ctx.enter_context(tc.tile_pool(name="xcp", bufs=2))
        up = ctx.enter_context(tc.tile_pool(name="up", bufs=1))
        vp = ctx.enter_context(tc.tile_pool(name="vp", bufs=1))
        tmp = ctx.enter_context(tc.tile_pool(name="tmp", bufs=4))
        sqp = ctx.enter_context(tc.tile_pool(name="sqp", bufs=12))
        stp = ctx.enter_context(tc.tile_pool(name="stp", bufs=1))
        bcp = ctx.enter_context(tc.tile_pool(name="bcp", bufs=2))
        psmm = ctx.enter_context(tc.tile_pool(name="psmm", bufs=4, space="PSUM"))
        psst = ctx.enter_context(tc.tile_pool(name="psst", bufs=2, space="PSUM"))

        # fp8 x first (phase_v consumes it immediately), then bf16 x
        # (consumed by fc, much later); chunked so matmuls can start as
        # soon as their chunk lands.
        x8s = []
        for j in range(EP):
            xj = x8p.tile([P, 2, TOK], F8, tag=f"x8{j}")
            eng = nc.gpsimd if j % 2 == 0 else nc.scalar
            eng.dma_start(xj[:], x8_d[j])
            x8s.append(xj)
        xbs = []
        for k in range(EO):
            xk = xbp.tile([P, TOK], BF16, tag=f"xb{k}")
            eng = nc.gpsimd if k % 2 == 0 else nc.scalar
            eng.dma_start(xk[:], xb_d[k * P:(k + 1) * P, :])
            xbs.append(xk)

        # --- constants (gpsimd queue keeps Sync free for the weight stream) ---
        bkv_t = consts.tile([P, EO], F32)
        nc.gpsimd.dma_start(bkv_t[:], bkv_d[:, :])
        c0_t = consts.tile([P, FO], F32)
        nc.gpsimd.dma_start(c0_t[:], c0_d[:, :])
        cb2_t = consts.tile([P, FO], F32)
        nc.gpsimd.dma_start(cb2_t[:], cb2_d[:, :])
        bmp_t = consts.tile([P, EO], F32)
        nc.gpsimd.dma_start(bmp_t[:], bmp_d[:, :])
        g2_t = consts.tile([P, EO], F32)
        nc.gpsimd.dma_start(g2_t[:], g2_d[:, :])
        b2_t = consts.tile([P, EO], F32)
        nc.gpsimd.dma_start(b2_t[:], b2_d[:, :])
        sc1_t = consts.tile([P, 1], F32)
        nc.gpsimd.dma_start(sc1_t[:], sc1_d[:, :])
        ones_col = consts.tile([P, 1], BF16)
        nc.vector.memset(ones_col[:], 1.0)
        eps_t = consts.tile([1, 1], F32)
        nc.vector.memset(eps_t[:], LN_EPS)

        # warm the PE (HAM clock gate) while input DMAs are in flight
        warm_rhs = consts.tile([P, T], BF16)
        nc.vector.memset(warm_rhs[:], 1.0)
        warm_ps = psst.tile([1, T], F32, tag="pss")
        for _ in range(40):
            nc.tensor.matmul(warm_ps[:], lhsT=ones_col[:], rhs=warm_rhs[:],
                             start=True, stop=True)

        def tsl(t):
            return slice(t * T, (t + 1) * T)

        def make_stats():
            """Quad-packed sum / sum-sq accumulators driven by chunk tiles."""
            ps_sum = psst.tile([1, T], F32, tag="pss", name="ps_sum")
            ps_sq = psst.tile([1, T], F32, tag="psq", name="ps_sq")
            st = {"ps_sum": ps_sum, "ps_sq": ps_sq, "vals": [], "sqs": [], "g": 0}

            def push(val):
                sq = sqp.tile([P, T], BF16, tag="sq")
                nc.vector.tensor_mul(out=sq[:], in0=val, in1=val)
                st["vals"].append(val)
                st["sqs"].append(sq[:])

            def emit_quad():
                v0, v1, v2, v3 = st["vals"][:4]
                del st["vals"][:4]
                q0, q1, q2, q3 = st["sqs"][:4]
                del st["sqs"][:4]
                a = sqp.tile([P, T], BF16, tag="sq")
                nc.vector.tensor_add(out=a[:], in0=v0, in1=v1)
                b = sqp.tile([P, T], BF16, tag="sq")
                nc.vector.tensor_add(out=b[:], in0=v2, in1=v3)
                c = sqp.tile([P, T], BF16, tag="sq")
                nc.vector.tensor_add(out=c[:], in0=a[:], in1=b[:])
                qa = sqp.tile([P, T], BF16, tag="sq")
                nc.vector.tensor_add(out=qa[:], in0=q0, in1=q1)
                qb = sqp.tile([P, T], BF16, tag="sq")
                nc.vector.tensor_add(out=qb[:], in0=q2, in1=q3)
                qc = sqp.tile([P, T], BF16, tag="sq")
                nc.vector.tensor_add(out=qc[:], in0=qa[:], in1=qb[:])
                g = st["g"]
                st["g"] += 1
                nc.tensor.matmul(st["ps_sum"][:], lhsT=ones_col[:], rhs=c[:],
                                 start=(g == 0), stop=(g == 3))
                nc.tensor.matmul(st["ps_sq"][:], lhsT=ones_col[:], rhs=qc[:],
                                 start=(g == 0), stop=(g == 3))

            st["push"] = push
            st["emit_quad"] = emit_quad
            return st

        # ---------- phase V: fp8 DoubleRow x@A, LN1 stats only ----------
        def phase_v(t, hook=None):
            stats = make_stats()
            for m in range(EO):
                if m in (5, 9, 13):
                    stats["emit_quad"]()
                a8t = a8p.tile([P, EP, 2, P], F8, tag="a8")
                nc.sync.dma_start(a8t[:], a8_d[m])
                ps = psmm.tile([P, T], F32, tag="ps")
                for j in range(EP):
                    nc.tensor.matmul(ps[:], lhsT=a8t[:, j],
                                     rhs=x8s[j][:, :, tsl(t)],
                                     start=(j == 0), stop=(j == EP - 1),
                                     perf_mode=DR)
                if m == 0 and hook is not None:
                    hook()
                r1c = r1p.tile([P, T], BF16, tag="r1")
                nc.scalar.activation(r1c[:], ps[:], AF.Identity,
                                     bias=bkv_t[:, m:m + 1],
                                     scale=sc1_t[:, 0:1])
                stats["push"](r1c[:])

            def finish():
                stats["emit_quad"]()
            return stats, finish

        def ln1_finalize(stats):
            """alpha = rsqrt(var + eps) -> [P,1,T] broadcast."""
            st = stp.tile([1, 3, T], F32, tag="st1")
            nc.vector.tensor_scalar_mul(st[:, 0, :], stats["ps_sum"][:], 1.0 / E)
            nc.vector.tensor_scalar_mul(st[:, 1, :], stats["ps_sq"][:], 1.0 / E)
            nc.vector.tensor_mul(out=st[:, 2, :], in0=st[:, 0, :], in1=st[:, 0, :])
            nc.vector.tensor_tensor(st[:, 1, :], st[:, 1, :], st[:, 2, :],
                                    ALU.subtract)                          # var
            nc.scalar.activation(st[:, 2, :], st[:, 1, :], AF.Sqrt,
                                 bias=eps_t[:], scale=1.0)
            nc.vector.reciprocal(out=st[:, 2, :], in_=st[:, 2, :])         # rstd
            bc = bcp.tile([P, 1, T], F32, tag="bc1")
            nc.gpsimd.partition_broadcast(bc[:], st[:, 2:3, :])
            return bc

        # ---------- phase FC: x@W1c, scale by alpha, gelu ----------
        def phase_fc(t, get_bc, hook=None):
            u = up.tile([P, FO, T], BF16, tag="u")
            for ma in range(FO):
                wt = wp.tile([P, EO, P], BF16, tag="w")
                nc.sync.dma_start(wt[:], w1_d[ma])
                ps = psmm.tile([P, T], F32, tag="ps")
                for k in range(EO):
                    nc.tensor.matmul(ps[:], lhsT=wt[:, k, :],
                                     rhs=xbs[k][:, tsl(t)],
                                     start=(k == 0), stop=(k == EO - 1))
                if ma == 0 and hook is not None:
                    hook()
                bc = get_bc()
                z1 = tmp.tile([P, T], BF16, tag="z1")
                if has_cb2:
                    t1 = tmp.tile([P, T], F32, tag="t1")
                    nc.scalar.activation(t1[:], ps[:], AF.Identity,
                                         bias=cb2_t[:, ma:ma + 1], scale=1.0)
                    nc.vector.tensor_mul(out=z1[:], in0=t1[:], in1=bc[:, 0, :])
                else:
                    nc.vector.tensor_mul(out=z1[:], in0=ps[:], in1=bc[:, 0, :])
                nc.scalar.activation(u[:, ma, :], z1[:], AF.Gelu,
                                     bias=c0_t[:, ma:ma + 1], scale=1.0)
            return u

        # ---------- phase MP: u@w_mproj + residual, LN2 stats ----------
        def phase_mp(t, u, hook=None):
            v2f = vp.tile([P, EO, T], BF16, tag="v")
            stats = make_stats()
            for mo in range(EO):
                if mo in (5, 9, 13):
                    stats["emit_quad"]()
                ps = psmm.tile([P, T], F32, tag="ps")
                for hc in range(HC):
                    wt = wp.tile([P, HCO, P], BF16, tag="w")
                    nc.sync.dma_start(wt[:], wmp_d[mo][:, hc * HCO:(hc + 1) * HCO, :])
                    for k in range(HCO):
                        nc.tensor.matmul(ps[:], lhsT=wt[:, k, :],
                                         rhs=u[:, hc * HCO + k, :],
                                         start=(hc == 0 and k == 0),
                                         stop=(hc == HC - 1 and k == HCO - 1))
                if mo == 0 and hook is not None:
                    hook()
                xc = xcp.tile([P, T], F32, tag="xc")
                nc.gpsimd.dma_start(xc[:], xf_d[mo * P:(mo + 1) * P, tsl(t)])
                nc.scalar.activation(v2f[:, mo, :], ps[:], AF.Identity,
                                     bias=bmp_t[:, mo:mo + 1], scale=1.0)
                nc.vector.tensor_add(out=v2f[:, mo, :], in0=v2f[:, mo, :],
                                     in1=xc[:])
                stats["push"](v2f[:, mo, :])

            def finish():
                stats["emit_quad"]()
            return v2f, stats, finish

        def ln2_finalize(stats):
            """slots: [0]=mean*rstd, [1]=rstd -> [P,2,T] broadcast."""
            st = stp.tile([1, 3, T], F32, tag="st2")
            nc.vector.tensor_scalar_mul(st[:, 0, :], stats["ps_sum"][:], 1.0 / E)
            nc.vector.tensor_scalar_mul(st[:, 1, :], stats["ps_sq"][:], 1.0 / E)
            nc.vector.tensor_mul(out=st[:, 2, :], in0=st[:, 0, :], in1=st[:, 0, :])
            nc.vector.tensor_tensor(st[:, 1, :], st[:, 1, :], st[:, 2, :],
                                    ALU.subtract)
            nc.scalar.activation(st[:, 2, :], st[:, 1, :], AF.Sqrt,
                                 bias=eps_t[:], scale=1.0)
            nc.vector.reciprocal(out=st[:, 2, :], in_=st[:, 2, :])         # rstd
            nc.vector.tensor_mul(out=st[:, 1, :], in0=st[:, 0, :], in1=st[:, 2, :])
            bc = bcp.tile([P, 2, T], F32, tag="bc2")
            nc.gpsimd.partition_broadcast(bc[:], st[:, 1:3, :])
            return bc

        # ---------- phase C: final normalize + output ----------
        def phase_c_out(t, v2f, bc, tail):
            dma_eng = nc.sync if tail else nc.gpsimd
            for m in range(EO):
                t1 = tmp.tile([P, T], F32, tag="t1")
                nc.vector.tensor_mul(out=t1[:], in0=v2f[:, m, :], in1=bc[:, 1, :])
                nc.vector.tensor_tensor(t1[:], t1[:], bc[:, 0, :], ALU.subtract)
                if tail:
                    nc.scalar.activation(t1[:], t1[:], AF.Identity,
                                         bias=b2_t[:, m:m + 1],
                                         scale=g2_t[:, m:m + 1])
                else:
                    nc.vector.tensor_scalar(t1[:], t1[:], g2_t[:, m:m + 1],
                                            b2_t[:, m:m + 1], ALU.mult, ALU.add)
                dma_eng.dma_start(out_d[m * P:(m + 1) * P, tsl(t)], t1[:])

        # Emission order: v0, v1, fc0, mp0, fc1, mp1, out1; each phase's
        # final stats MMs and LN finalize chains are deferred into the next
        # phase's first block so the PE never waits on DVE chains.
        state = {}
        s0, fin0 = phase_v(0)

        def hook_v1():
            fin0()
            state["bc10"] = ln1_finalize(s0)

        s1, fin1 = phase_v(1, hook=hook_v1)

        def hook_fc0():
            fin1()
            state["bc11"] = ln1_finalize(s1)

        u0 = phase_fc(0, lambda: state["bc10"], hook=hook_fc0)
        v0, s20, fin20 = phase_mp(0, u0)

        def hook_fc1():
            fin20()
            bc20 = ln2_finalize(s20)
            phase_c_out(0, v0, bc20, tail=False)

        u1 = phase_fc(1, lambda: state["bc11"], hook=hook_fc1)
        v1, s21, fin21 = phase_mp(1, u1)
        fin21()
        bc21 = ln2_finalize(s21)
        phase_c_out(1, v1, bc21, tail=True)

    nc.compile()
    return nc


def _get_nc(has_cb2: bool):
    if has_cb2 not in _CACHED_NC:
        _CACHED_NC[has_cb2] = _build(has_cb2)
    return _CACHED_NC[has_cb2]


def _pow2_scale(amax):
    if amax <= 0:
        return 1.0
    return float(2.0 ** np.floor(np.log2(F8MAX / amax)))


def _prep_inputs(x, w_kv, b_kv, w_fc, b_fc, w_mproj, b_mproj,
                 ln1_g, ln1_b, ln2_g, ln2_b):
    """Host-side fold + shard + retile. Returns (per-core input maps, has_cb2)."""
    bf = ml_dtypes.bfloat16
    f8 = ml_dtypes.float8_e4m3
    x_flat = np.ascontiguousarray(np.asarray(x, dtype=np.float32).reshape(B * S, E))
    w_kv = np.asarray(w_kv, np.float64)
    b_kv = np.asarray(b_kv, np.float64)
    w_fc = np.asarray(w_fc, np.float64)
    b_fc = np.asarray(b_fc, np.float64)
    g1 = np.asarray(ln1_g, np.float64)
    b1 = np.asarray(ln1_b, np.float64)

    # A = I + w_kv ; centered fold W1c = A @ (I - 11^T/E) @ diag(g1) @ w_fc
    A = w_kv.copy()
    A[np.diag_indices(E)] += 1.0
    Wg = w_fc * g1[:, None]
    Ac = A - A.sum(axis=1, keepdims=True) / E       # A @ C
    W1c = (Ac @ Wg).astype(np.float32)
    cb2 = ((b_kv - b_kv.mean()) @ Wg).astype(np.float32)     # b_kv @ C @ Wg
    c0 = (b1 @ w_fc + b_fc).astype(np.float32)
    has_cb2 = bool(np.any(cb2 != 0.0))

    # fp8 quantization of A (stats path) and x
    s_A = _pow2_scale(np.abs(A).max())
    A8 = np.clip(A * s_A, -F8MAX, F8MAX).astype(f8)
    s_x = _pow2_scale(np.abs(x_flat).max())
    # a8 layout: [m, p, j, i, c] = A8[(2j+i)*128+p, m*128+c]
    a8 = np.ascontiguousarray(
        A8.reshape(EP, 2, P, EO, P).transpose(3, 2, 0, 1, 4))

    # weights: [in, out] -> [m, p, o, c] tiles, in = o*128+p, out = m*128+c
    def retile(w, io, oo):
        w = np.asarray(w, dtype=np.float32).reshape(io, P, oo, P)
        return np.ascontiguousarray(w.transpose(2, 1, 0, 3)).astype(bf)

    w1_t = retile(W1c, EO, FO)
    wmp_t = retile(w_mproj, FO, EO)

    def p2d(v):  # [n*P] -> [P, n] with chunk o in column o
        v = np.asarray(v, np.float32)
        return np.ascontiguousarray(v.reshape(-1, P).T)

    shared = {
        "a8": a8, "w1": w1_t, "wmp": wmp_t,
        "bkv": p2d(b_kv.astype(np.float32)), "c0": p2d(c0), "cb2": p2d(cb2),
        "bmp": p2d(b_mproj),
        "g2": p2d(ln2_g), "b2": p2d(ln2_b),
        "sc1": np.full((P, 1), 1.0 / (s_A * s_x), np.float32),
    }
    in_maps = []
    for c in range(NCORES):
        xT = np.ascontiguousarray(x_flat[c * TOK:(c + 1) * TOK].T)  # [E, TOK] f32
        x8c = np.clip(xT * s_x, -F8MAX, F8MAX).astype(f8)           # [E, TOK]
        # x8 layout: [j, p, i, t] = x8c[(2j+i)*128+p, t]
        x8c = np.ascontiguousarray(
            x8c.reshape(EP, 2, P, TOK).transpose(0, 2, 1, 3))
        in_maps.append({"xf": xT, "xb": xT.astype(bf), "x8": x8c, **shared})
    return in_maps, has_cb2


def _run(inputs, trace=False):
    in_maps, has_cb2 = _prep_inputs(
        inputs["x"], inputs["w_kv"], inputs["b_kv"], inputs["w_fc"],
        inputs["b_fc"], inputs["w_mproj"], inputs["b_mproj"],
        inputs["ln1_g"], inputs["ln1_b"], inputs["ln2_g"], inputs["ln2_b"])
    nc = _get_nc(has_cb2)
    res = run_bass_kernel_spmd(nc, in_maps, core_ids=list(range(NCORES)),
                               trace=trace)
    outs = [np.asarray(res.results[c]["out"], dtype=np.float32).T
            for c in range(NCORES)]
    full = np.concatenate(outs, axis=0).reshape(B, S, E)
    return full, res


def kernel(**inputs) -> np.ndarray:
    out, _ = _run(inputs, trace=False)
    return out


# revision 22
# speedup vs baseline: 1.2406x; 1.0084x over previous
"""Trainium2 Bass kernel for nn_Block_68719476955 (dense transformer block).

Math: with H=1 the attention softmax is over a singleton axis, so
attn_prob == 1.0 exactly and the whole attention reduces to
x @ w_kv + b_kv (w_attn / b_attn / mask do not affect the output).

Folded main path: with A = I + w_kv and C = I - 11^T/E (mean-centering),
    LN1(x@A + b_kv) @ w_fc  ==  rstd ⊙ (x @ W1c + cb2) + c0
where W1c = A @ C @ (diag(g1) w_fc) is precomputed on host, cb2/c0 are
constant vectors, and rstd is the per-token LN1 inverse std.  So the
E x E kv matmul disappears from the main bf16 path; it is needed only
to produce the LN1 *variance*, which tolerates fp8: that matmul runs as
fp8e4 (TRN e4m3) DoubleRow matmuls at ~2x column rate.

Block per token row (E=2048):
    r1   = x @ A + b_kv              # fp8 DoubleRow, feeds only mean/var
    alpha= rsqrt(var(r1) + eps)
    u    = gelu(alpha*(x @ W1c + cb2) + c0)     # bf16 matmul
    v    = u @ w_mproj + b_mproj                # bf16 matmul
    out  = LN(v + x) * g2 + b2

Distribution: pure data-parallel over the 8192 tokens across 8 cores
(1024 tokens/core), full weights on every core, no collectives.

Device layout: feature-major ("transposed") activations [E, tokens];
every matmul has the weight block stationary.  LayerNorm feature
reductions run on the TensorEngine as ones-vector matmuls over
quad-summed chunks; per-token stats broadcast back across partitions
via gpsimd.partition_broadcast.
"""

import numpy as np
import ml_dtypes
from contextlib import ExitStack

import concourse.bacc as bacc
import concourse.mybir as mybir
import concourse.tile as tile
from concourse.bass_utils import run_bass_kernel_spmd

P = 128
B, S, E = 4, 2048, 2048
H4 = 4 * E                 # 8192 mlp hidden
NCORES = 8
TOK = (B * S) // NCORES    # 1024 tokens per core
T = 512                    # token tile (2 per core)
NT = TOK // T
EO = E // P                # 16
EP = EO // 2               # 8 fp8 k-pairs
FO = H4 // P               # 64
HC = 4                     # hidden chunks for the mlp (2048 features each)
HCO = FO // HC             # 16 m-blocks per hidden chunk
LN_EPS = 1e-5
F8MAX = 240.0

F32 = mybir.dt.float32
BF16 = mybir.dt.bfloat16
F8 = mybir.dt.float8e4
AF = mybir.ActivationFunctionType
ALU = mybir.AluOpType
DR = mybir.MatmulPerfMode.DoubleRow

_CACHED_NC = {}


def _build(has_cb2: bool):
    nc = bacc.Bacc(None, target_bir_lowering=False)

    xf_d = nc.dram_tensor("xf", [E, TOK], F32, kind="ExternalInput")
    xb_d = nc.dram_tensor("xb", [E, TOK], BF16, kind="ExternalInput")
    x8_d = nc.dram_tensor("x8", [EP, P, 2, TOK], F8, kind="ExternalInput")
    a8_d = nc.dram_tensor("a8", [EO, P, EP, 2, P], F8, kind="ExternalInput")
    w1_d = nc.dram_tensor("w1", [FO, P, EO, P], BF16, kind="ExternalInput")
    wmp_d = nc.dram_tensor("wmp", [EO, P, FO, P], BF16, kind="ExternalInput")
    bkv_d = nc.dram_tensor("bkv", [P, EO], F32, kind="ExternalInput")
    c0_d = nc.dram_tensor("c0", [P, FO], F32, kind="ExternalInput")
    cb2_d = nc.dram_tensor("cb2", [P, FO], F32, kind="ExternalInput")
    bmp_d = nc.dram_tensor("bmp", [P, EO], F32, kind="ExternalInput")
    g2_d = nc.dram_tensor("g2", [P, EO], F32, kind="ExternalInput")
    b2_d = nc.dram_tensor("b2", [P, EO], F32, kind="ExternalInput")
    sc1_d = nc.dram_tensor("sc1", [P, 1], F32, kind="ExternalInput")
    out_d = nc.dram_tensor("out", [E, TOK], F32, kind="ExternalOutput")

    with tile.TileContext(nc) as tc, ExitStack() as ctx:
        consts = ctx.enter_context(tc.tile_pool(name="consts", bufs=1))
        xbp = ctx.enter_context(tc.tile_pool(name="xbp", bufs=1))
        x8p = ctx.enter_context(tc.tile_pool(name="x8p", bufs=1))
        a8p = ctx.enter_context(tc.tile_pool(name="a8p", bufs=2))
        wp = ctx.enter_context(tc.tile_pool(name="wp", bufs=3))
        r1p = ctx.enter_context(tc.tile_pool(name="r1p", bufs=10))
        xcp = ctx.enter_context(tc.tile_pool(name="xcp", bufs=2))
        up = ctx.enter_context(tc.tile_pool(name="up", bufs=1))
        vp = ctx.enter_context(tc.tile_pool(name="vp", bufs=1))
        tmp = ctx.enter_context(tc.tile_pool(name="tmp", bufs=4))
        sqp = ctx.enter_context(tc.tile_pool(name="sqp", bufs=12))
        stp = ctx.enter_context(tc.tile_pool(name="stp", bufs=1))
        bcp = ctx.enter_context(tc.tile_pool(name="bcp", bufs=2))
        psmm = ctx.enter_context(tc.tile_pool(name="psmm", bufs=4, space="PSUM"))
        psst = ctx.enter_context(tc.tile_pool(name="psst", bufs=2, space="PSUM"))

        # fp8 x first (phase_v consumes it immediately), then bf16 x
        # (consumed by fc, much later); chunked so matmuls can start as
        # soon as their chunk lands.
        x8s = []
        for j in range(EP):
            xj = x8p.tile([P, 2, TOK], F8, tag=f"x8{j}")
            eng = nc.gpsimd if j % 2 == 0 else nc.scalar
            eng.dma_start(xj[:], x8_d[j])
            x8s.append(xj)
        xbs = []
        for k in range(EO):
            xk = xbp.tile([P, TOK], BF16, tag=f"xb{k}")
            eng = nc.gpsimd if k % 2 == 0 else nc.scalar
            eng.dma_start(xk[:], xb_d[k * P:(k + 1) * P, :])
            xbs.append(xk)

        # --- constants (gpsimd queue keeps Sync free for the weight stream) ---
        bkv_t = consts.tile([P, EO], F32)
        nc.gpsimd.dma_start(bkv_t[:], bkv_d[:, :])
        c0_t = consts.tile([P, FO], F32)
        nc.gpsimd.dma_start(c0_t[:], c0_d[:, :])
        cb2_t = consts.tile([P, FO], F32)
        nc.gpsimd.dma_start(cb2_t[:], cb2_d[:, :])
        bmp_t = consts.tile([P, EO], F32)
        nc.gpsimd.dma_start(bmp_t[:], bmp_d[:, :])
        g2_t = consts.tile([P, EO], F32)
        nc.gpsimd.dma_start(g2_t[:], g2_d[:, :])
        b2_t = consts.tile([P, EO], F32)
        nc.gpsimd.dma_start(b2_t[:], b2_d[:, :])
        sc1_t = consts.tile([P, 1], F32)
        nc.gpsimd.dma_start(sc1_t[:], sc1_d[:, :])
        ones_col = consts.tile([P, 1], BF16)
        nc.vector.memset(ones_col[:], 1.0)
        eps_t = consts.tile([1, 1], F32)
        nc.vector.memset(eps_t[:], LN_EPS)

        # warm the PE (HAM clock gate) while input DMAs are in flight
        warm_rhs = consts.tile([P, T], BF16)
        nc.vector.memset(warm_rhs[:], 1.0)
        warm_ps = psst.tile([1, T], F32, tag="pss")
        for _ in range(64):
            nc.tensor.matmul(warm_ps[:], lhsT=ones_col[:], rhs=warm_rhs[:],
                             start=True, stop=True)

        def tsl(t):
            return slice(t * T, (t + 1) * T)

        def make_stats():
            """Quad-packed sum / sum-sq accumulators driven by chunk tiles."""
            ps_sum = psst.tile([1, T], F32, tag="pss", name="ps_sum")
            ps_sq = psst.tile([1, T], F32, tag="psq", name="ps_sq")
            st = {"ps_sum": ps_sum, "ps_sq": ps_sq, "vals": [], "sqs": [], "g": 0}

            def push(val):
                sq = sqp.tile([P, T], BF16, tag="sq")
                nc.vector.tensor_mul(out=sq[:], in0=val, in1=val)
                st["vals"].append(val)
                st["sqs"].append(sq[:])

            def emit_quad():
                v0, v1, v2, v3 = st["vals"][:4]
                del st["vals"][:4]
                q0, q1, q2, q3 = st["sqs"][:4]
                del st["sqs"][:4]
                a = sqp.tile([P, T], BF16, tag="sq")
                nc.vector.tensor_add(out=a[:], in0=v0, in1=v1)
                b = sqp.tile([P, T], BF16, tag="sq")
                nc.vector.tensor_add(out=b[:], in0=v2, in1=v3)
                c = sqp.tile([P, T], BF16, tag="sq")
                nc.vector.tensor_add(out=c[:], in0=a[:], in1=b[:])
                qa = sqp.tile([P, T], BF16, tag="sq")
                nc.vector.tensor_add(out=qa[:], in0=q0, in1=q1)
                qb = sqp.tile([P, T], BF16, tag="sq")
                nc.vector.tensor_add(out=qb[:], in0=q2, in1=q3)
                qc = sqp.tile([P, T], BF16, tag="sq")
                nc.vector.tensor_add(out=qc[:], in0=qa[:], in1=qb[:])
                g = st["g"]
                st["g"] += 1
                nc.tensor.matmul(st["ps_sum"][:], lhsT=ones_col[:], rhs=c[:],
                                 start=(g == 0), stop=(g == 3))
                nc.tensor.matmul(st["ps_sq"][:], lhsT=ones_col[:], rhs=qc[:],
                                 start=(g == 0), stop=(g == 3))

            st["push"] = push
            st["emit_quad"] = emit_quad
            return st

        # ---------- phase V: fp8 DoubleRow x@A, LN1 stats only ----------
        # processes BOTH token tiles per A-chunk load (A DMA'd once).
        def phase_v_both():
            st0 = make_stats()
            st1 = make_stats()
            for m in range(EO):
                if m in (4, 8, 12):
                    st0["emit_quad"]()
                    st1["emit_quad"]()
                a8t = a8p.tile([P, EP, 2, P], F8, tag="a8")
                nc.sync.dma_start(a8t[:], a8_d[m])
                for t in range(NT):
                    ps = psmm.tile([P, T], F32, tag="ps")
                    for j in range(EP):
                        nc.tensor.matmul(ps[:], lhsT=a8t[:, j],
                                         rhs=x8s[j][:, :, tsl(t)],
                                         start=(j == 0), stop=(j == EP - 1),
                                         perf_mode=DR)
                    r1c = r1p.tile([P, T], BF16, tag="r1")
                    nc.scalar.activation(r1c[:], ps[:], AF.Identity,
                                         bias=bkv_t[:, m:m + 1],
                                         scale=sc1_t[:, 0:1])
                    (st0 if t == 0 else st1)["push"](r1c[:])

            def finish():
                st0["emit_quad"]()
                st1["emit_quad"]()
            return st0, st1, finish

        def ln1_finalize(stats):
            """alpha = rsqrt(var + eps) -> [P,1,T] broadcast."""
            st = stp.tile([1, 3, T], F32, tag="st1")
            nc.vector.tensor_scalar_mul(st[:, 0, :], stats["ps_sum"][:], 1.0 / E)
            nc.vector.tensor_scalar_mul(st[:, 1, :], stats["ps_sq"][:], 1.0 / E)
            nc.vector.tensor_mul(out=st[:, 2, :], in0=st[:, 0, :], in1=st[:, 0, :])
            nc.vector.tensor_tensor(st[:, 1, :], st[:, 1, :], st[:, 2, :],
                                    ALU.subtract)                          # var
            nc.scalar.activation(st[:, 2, :], st[:, 1, :], AF.Sqrt,
                                 bias=eps_t[:], scale=1.0)
            nc.vector.reciprocal(out=st[:, 2, :], in_=st[:, 2, :])         # rstd
            bc = bcp.tile([P, 1, T], F32, tag="bc1")
            nc.gpsimd.partition_broadcast(bc[:], st[:, 2:3, :])
            return bc

        # ---------- phase FC: x@W1c, scale by alpha, gelu ----------
        def phase_fc(t, get_bc, hook=None, block_cb=None):
            u = up.tile([P, FO, T], BF16, tag="u")
            for ma in range(FO):
                wt = wp.tile([P, EO, P], BF16, tag="w")
                nc.sync.dma_start(wt[:], w1_d[ma])
                ps = psmm.tile([P, T], F32, tag="ps")
                for k in range(EO):
                    nc.tensor.matmul(ps[:], lhsT=wt[:, k, :],
                                     rhs=xbs[k][:, tsl(t)],
                                     start=(k == 0), stop=(k == EO - 1))
                if ma == 0 and hook is not None:
                    hook()
                if block_cb is not None:
                    block_cb(ma)
                bc = get_bc()
                z1 = tmp.tile([P, T], BF16, tag="z1")
                if has_cb2:
                    t1 = tmp.tile([P, T], F32, tag="t1")
                    nc.scalar.activation(t1[:], ps[:], AF.Identity,
                                         bias=cb2_t[:, ma:ma + 1], scale=1.0)
                    nc.vector.tensor_mul(out=z1[:], in0=t1[:], in1=bc[:, 0, :])
                else:
                    nc.vector.tensor_mul(out=z1[:], in0=ps[:], in1=bc[:, 0, :])
                nc.scalar.activation(u[:, ma, :], z1[:], AF.Gelu,
                                     bias=c0_t[:, ma:ma + 1], scale=1.0)
            return u

        # ---------- phase MP: u@w_mproj + residual, LN2 stats ----------
        def phase_mp(t, u, hook=None):
            v2f = vp.tile([P, EO, T], BF16, tag="v")
            stats = make_stats()
            for mo in range(EO):
                if mo in (5, 9, 13):
                    stats["emit_quad"]()
                ps = psmm.tile([P, T], F32, tag="ps")
                for hc in range(HC):
                    wt = wp.tile([P, HCO, P], BF16, tag="w")
                    nc.sync.dma_start(wt[:], wmp_d[mo][:, hc * HCO:(hc + 1) * HCO, :])
                    for k in range(HCO):
                        nc.tensor.matmul(ps[:], lhsT=wt[:, k, :],
                                         rhs=u[:, hc * HCO + k, :],
                                         start=(hc == 0 and k == 0),
                                         stop=(hc == HC - 1 and k == HCO - 1))
                if mo == 0 and hook is not None:
                    hook()
                xc = xcp.tile([P, T], F32, tag="xc")
                nc.gpsimd.dma_start(xc[:], xf_d[mo * P:(mo + 1) * P, tsl(t)])
                nc.scalar.activation(v2f[:, mo, :], ps[:], AF.Identity,
                                     bias=bmp_t[:, mo:mo + 1], scale=1.0)
                nc.vector.tensor_add(out=v2f[:, mo, :], in0=v2f[:, mo, :],
                                     in1=xc[:])
                stats["push"](v2f[:, mo, :])

            def finish():
                stats["emit_quad"]()
            return v2f, stats, finish

        def ln2_finalize(stats):
            """slots: [0]=mean*rstd, [1]=rstd -> [P,2,T] broadcast."""
            st = stp.tile([1, 3, T], F32, tag="st2")
            nc.vector.tensor_scalar_mul(st[:, 0, :], stats["ps_sum"][:], 1.0 / E)
            nc.vector.tensor_scalar_mul(st[:, 1, :], stats["ps_sq"][:], 1.0 / E)
            nc.vector.tensor_mul(out=st[:, 2, :], in0=st[:, 0, :], in1=st[:, 0, :])
            nc.vector.tensor_tensor(st[:, 1, :], st[:, 1, :], st[:, 2, :],
                                    ALU.subtract)
            nc.scalar.activation(st[:, 2, :], st[:, 1, :], AF.Sqrt,
                                 bias=eps_t[:], scale=1.0)
            nc.vector.reciprocal(out=st[:, 2, :], in_=st[:, 2, :])         # rstd
            nc.vector.tensor_mul(out=st[:, 1, :], in0=st[:, 0, :], in1=st[:, 2, :])
            bc = bcp.tile([P, 2, T], F32, tag="bc2")
            nc.gpsimd.partition_broadcast(bc[:], st[:, 1:3, :])
            return bc

        # ---------- phase C: final normalize + output ----------
        def phase_c_chunk(t, v2f, bc, m, tail):
            # tail: offload the last chunks' elementwise work to gpsimd so
            # DVE and gpsimd drain the post-matmul tail in parallel.
            eng = nc.gpsimd if (tail and m >= 12) else nc.vector
            t1 = tmp.tile([P, T], F32, tag="t1")
            eng.tensor_mul(out=t1[:], in0=v2f[:, m, :], in1=bc[:, 1, :])
            eng.tensor_tensor(t1[:], t1[:], bc[:, 0, :], ALU.subtract)
            if tail:
                nc.scalar.activation(t1[:], t1[:], AF.Identity,
                                     bias=b2_t[:, m:m + 1],
                                     scale=g2_t[:, m:m + 1])
            else:
                nc.vector.tensor_scalar(t1[:], t1[:], g2_t[:, m:m + 1],
                                        b2_t[:, m:m + 1], ALU.mult, ALU.add)
            dma_eng = nc.sync if tail else (nc.gpsimd if m % 2 == 0 else nc.scalar)
            dma_eng.dma_start(out_d[m * P:(m + 1) * P, tsl(t)], t1[:])

        def phase_c_out(t, v2f, bc, tail):
            for m in range(EO):
                phase_c_chunk(t, v2f, bc, m, tail)

        # Emission order: v(both), fc0, mp0, fc1, mp1, out1; each phase's
        # final stats MMs and LN finalize chains are deferred into the next
        # phase's first block so the PE never waits on DVE chains; tile 0's
        # output normalize is spread across fc1's blocks so the DVE queue
        # never backs up ahead of fc1's psum evacuations.
        state = {}
        s0, s1, finv = phase_v_both()

        def hook_fc0():
            finv()
            state["bc10"] = ln1_finalize(s0)
            state["bc11"] = ln1_finalize(s1)

        u0 = phase_fc(0, lambda: state["bc10"], hook=hook_fc0)
        v0, s20, fin20 = phase_mp(0, u0)

        def hook_fc1():
            fin20()
            state["bc20"] = ln2_finalize(s20)

        def cb_out0(ma):
            # 2 output chunks of tile 0 per fc1 block, blocks 1..8
            if 1 <= ma <= 8:
                for m in (2 * ma - 2, 2 * ma - 1):
                    phase_c_chunk(0, v0, state["bc20"], m, tail=False)

        u1 = phase_fc(1, lambda: state["bc11"], hook=hook_fc1, block_cb=cb_out0)
        v1, s21, fin21 = phase_mp(1, u1)
        fin21()
        bc21 = ln2_finalize(s21)
        phase_c_out(1, v1, bc21, tail=True)

    nc.compile()
    return nc


def _get_nc(has_cb2: bool):
    if has_cb2 not in _CACHED_NC:
        _CACHED_NC[has_cb2] = _build(has_cb2)
    return _CACHED_NC[has_cb2]


def _pow2_scale(amax):
    if amax <= 0:
        return 1.0
    return float(2.0 ** np.floor(np.log2(F8MAX / amax)))


def _prep_inputs(x, w_kv, b_kv, w_fc, b_fc, w_mproj, b_mproj,
                 ln1_g, ln1_b, ln2_g, ln2_b):
    """Host-side fold + shard + retile. Returns (per-core input maps, has_cb2)."""
    bf = ml_dtypes.bfloat16
    f8 = ml_dtypes.float8_e4m3
    x_flat = np.ascontiguousarray(np.asarray(x, dtype=np.float32).reshape(B * S, E))
    w_kv = np.asarray(w_kv, np.float64)
    b_kv = np.asarray(b_kv, np.float64)
    w_fc = np.asarray(w_fc, np.float64)
    b_fc = np.asarray(b_fc, np.float64)
    g1 = np.asarray(ln1_g, np.float64)
    b1 = np.asarray(ln1_b, np.float64)

    # A = I + w_kv ; centered fold W1c = A @ (I - 11^T/E) @ diag(g1) @ w_fc
    A = w_kv.copy()
    A[np.diag_indices(E)] += 1.0
    Wg = w_fc * g1[:, None]
    Ac = A - A.sum(axis=1, keepdims=True) / E       # A @ C
    W1c = (Ac @ Wg).astype(np.float32)
    cb2 = ((b_kv - b_kv.mean()) @ Wg).astype(np.float32)     # b_kv @ C @ Wg
    c0 = (b1 @ w_fc + b_fc).astype(np.float32)
    has_cb2 = bool(np.any(cb2 != 0.0))

    # fp8 quantization of A (stats path) and x
    s_A = _pow2_scale(np.abs(A).max())
    A8 = np.clip(A * s_A, -F8MAX, F8MAX).astype(f8)
    s_x = _pow2_scale(np.abs(x_flat).max())
    # a8 layout: [m, p, j, i, c] = A8[(2j+i)*128+p, m*128+c]
    a8 = np.ascontiguousarray(
        A8.reshape(EP, 2, P, EO, P).transpose(3, 2, 0, 1, 4))

    # weights: [in, out] -> [m, p, o, c] tiles, in = o*128+p, out = m*128+c
    def retile(w, io, oo):
        w = np.asarray(w, dtype=np.float32).reshape(io, P, oo, P)
        return np.ascontiguousarray(w.transpose(2, 1, 0, 3)).astype(bf)

    w1_t = retile(W1c, EO, FO)
    wmp_t = retile(w_mproj, FO, EO)

    def p2d(v):  # [n*P] -> [P, n] with chunk o in column o
        v = np.asarray(v, np.float32)
        return np.ascontiguousarray(v.reshape(-1, P).T)

    shared = {
        "a8": a8, "w1": w1_t, "wmp": wmp_t,
        "bkv": p2d(b_kv.astype(np.float32)), "c0": p2d(c0), "cb2": p2d(cb2),
        "bmp": p2d(b_mproj),
        "g2": p2d(ln2_g), "b2": p2d(ln2_b),
        "sc1": np.full((P, 1), 1.0 / (s_A * s_x), np.float32),
    }
    in_maps = []
    for c in range(NCORES):
        xT = np.ascontiguousarray(x_flat[c * TOK:(c + 1) * TOK].T)  # [E, TOK] f32
        x8c = np.clip(xT * s_x, -F8MAX, F8MAX).astype(f8)           # [E, TOK]
        # x8 layout: [j, p, i, t] = x8c[(2j+i)*128+p, t]
        x8c = np.ascontiguousarray(
            x8c.reshape(EP, 2, P, TOK).transpose(0, 2, 1, 3))
        in_maps.append({"xf": xT, "xb": xT.astype(bf), "x8": x8c, **shared})
    return in_maps, has_cb2


def _run(inputs, trace=False):
    in_maps, has_cb2 = _prep_inputs(
        inputs["x"], inputs["w_kv"], inputs["b_kv"], inputs["w_fc"],
        inputs["b_fc"], inputs["w_mproj"], inputs["b_mproj"],
        inputs["ln1_g"], inputs["ln1_b"], inputs["ln2_g"], inputs["ln2_b"])
    nc = _get_nc(has_cb2)
    res = run_bass_kernel_spmd(nc, in_maps, core_ids=list(range(NCORES)),
                               trace=trace)
    outs = [np.asarray(res.results[c]["out"], dtype=np.float32).T
            for c in range(NCORES)]
    full = np.concatenate(outs, axis=0).reshape(B, S, E)
    return full, res


def kernel(**inputs) -> np.ndarray:
    out, _ = _run(inputs, trace=False)
    return out
